# revision 1
# baseline (speedup 1.0000x reference)
"""Multi-head attention (B=4, S=2048, d_model=1024, H=16) on 8 TRN2 NeuronCores.

Sharding: tensor-parallel over heads x data-parallel over batch.
Core c handles batch b=c//2 and head group g=c%2 (8 heads = 512 of the
1024 d_model columns of W_Q/W_K/W_V, and 512 rows of W_O). Each core
produces a partial output Y_partial[b] = O_g @ W_O[g-rows, :]; the host
sums the two partials per batch.

Device-side dataflow per core (all matmul operands fp16, accum fp32):
  - inputs arrive pre-transposed: X^T in [dmodel, token] layout
  - k^T, q^T = W^T X^T         (lhsT = W chunk, rhs = X^T chunk)
  - v = X @ W_V   in [token, head-dim] layout, with a ones column
  - per head h, per 128-ktok block: scores^T = k^T.T q^T  -> PSUM
    exp (scale=1/8 fused into the activation)             -> P^T fp16
    out^T_ext += [v_h | 1].T @ P^T   (row 64 = softmax denominator)
  - out^T / denominator -> O^T; Y_partial = O @ W_O slice -> DRAM
"""

import numpy as np

B = 4
S = 2048
D = 1024
H = 16
DK = 64
NCORES = 8
HPC = 8          # heads per core
GCOLS = 512      # d_model columns per head group
QB = 512         # q-token block (PSUM bank free dim)
NQB = S // QB    # 4
NKB = S // 128   # 16 k-token blocks
NC_CHUNKS = D // 128  # 8 contraction chunks

_prog_cache = {}


def build_program(reps=1):
    """Build + compile the SPMD program. Cached per `reps`."""
    if reps in _prog_cache:
        return _prog_cache[reps]

    import concourse.bacc as bacc
    import concourse.mybir as mybir
    from concourse.tile import TileContext

    f16 = mybir.dt.float16
    f32 = mybir.dt.float32
    EXP = mybir.ActivationFunctionType.Exp

    nc = bacc.Bacc("TRN2", target_bir_lowering=False, debug=False,
                   num_devices=NCORES)

    # DRAM parameters (per-core shards, pre-laid-out on host)
    qt_d = nc.dram_tensor("qt", [128, NC_CHUNKS, S], f16, kind="ExternalInput").ap()
    kt_d = nc.dram_tensor("kt", [128, NC_CHUNKS, S], f16, kind="ExternalInput").ap()
    vt_d = nc.dram_tensor("vt", [128, NC_CHUNKS, S], f16, kind="ExternalInput").ap()
    wq_d = nc.dram_tensor("wq", [128, NC_CHUNKS, GCOLS], f16, kind="ExternalInput").ap()
    wk_d = nc.dram_tensor("wk", [128, NC_CHUNKS, GCOLS], f16, kind="ExternalInput").ap()
    wv_d = nc.dram_tensor("wv", [128, NC_CHUNKS, GCOLS], f16, kind="ExternalInput").ap()
    wo_d = nc.dram_tensor("wo", [128, 4, D], f16, kind="ExternalInput").ap()
    yp_d = nc.dram_tensor("yp", [S, D], f32, kind="ExternalOutput").ap()

    with TileContext(nc) as tc:
        with tc.tile_pool(name="weights", bufs=1) as wpool, \
             tc.tile_pool(name="xt", bufs=2) as xtpool, \
             tc.tile_pool(name="proj", bufs=1) as projpool, \
             tc.tile_pool(name="work", bufs=2) as workpool:

            for rep in range(reps):
                # ---- load weights (resident) ----
                wq_sb = wpool.tile([128, NC_CHUNKS, GCOLS], f16, name="wq_sb", tag="wq")
                wk_sb = wpool.tile([128, NC_CHUNKS, GCOLS], f16, name="wk_sb", tag="wk")
                wv_sb = wpool.tile([128, NC_CHUNKS, GCOLS], f16, name="wv_sb", tag="wv")
                wo_sb = wpool.tile([128, 4, D], f16, name="wo_sb", tag="wo")
                nc.sync.dma_start(out=wq_sb[:], in_=wq_d[:])
                nc.sync.dma_start(out=wk_sb[:], in_=wk_d[:])
                nc.sync.dma_start(out=wv_sb[:], in_=wv_d[:])
                nc.sync.dma_start(out=wo_sb[:], in_=wo_d[:])

                # ---- projection outputs (resident) ----
                # kT/qT: [dk-on-partitions, token] per head pair; chunk m holds
                # heads 2m (partitions 0:64) and 2m+1 (partitions 64:128)
                kT_sb = projpool.tile([128, 4, S], f16, name="kT_sb", tag="kT")
                qT_sb = projpool.tile([128, 4, S], f16, name="qT_sb", tag="qT")
                # v: [token-on-partitions, head, dim(+ones)] per 128-token block
                v_sb = projpool.tile([128, NKB, HPC, 72], f16, name="v_sb", tag="v")
                oT_sb = projpool.tile([128, 4, S], f16, name="oT_sb", tag="oT")

                # ---- projections ----
                with tc.tile_pool(name="pps", bufs=3, space="PSUM") as pps:
                    for which, (x_d, w_sb, dst) in enumerate(
                        ((kt_d, wk_sb, kT_sb), (qt_d, wq_sb, qT_sb))):
                        xt_sb = xtpool.tile([128, NC_CHUNKS, S], f16,
                                            name="xt_sb", tag="xt")
                        nc.sync.dma_start(out=xt_sb[:], in_=x_d[:])
                        # out^T tile [hd 128, tok 512] = sum_c W[c]^T.T @ X^T[c]
                        for m in range(4):
                            for n in range(NQB):
                                ps = pps.tile([128, QB], f32, name="proj_ps",
                                              tag="pps")
                                for c in range(NC_CHUNKS):
                                    nc.tensor.matmul(
                                        ps[:],
                                        w_sb[:, c, m * 128:(m + 1) * 128],
                                        xt_sb[:, c, n * QB:(n + 1) * QB],
                                        start=(c == 0), stop=(c == NC_CHUNKS - 1))
                                nc.vector.tensor_copy(
                                    dst[:, m, n * QB:(n + 1) * QB], ps[:])

                    # V projection -> v_sb [tok, head, d] with ones column
                    xt_sb = xtpool.tile([128, NC_CHUNKS, S], f16,
                                        name="xt_sb", tag="xt")
                    nc.sync.dma_start(out=xt_sb[:], in_=vt_d[:])
                    for kb in range(NKB):
                        nc.vector.memset(v_sb[:, kb, :, :], 1.0)
                    for kb in range(NKB):
                        ps = pps.tile([128, GCOLS], f32, name="vproj_ps", tag="pps")
                        for c in range(NC_CHUNKS):
                            nc.tensor.matmul(
                                ps[:],
                                xt_sb[:, c, kb * 128:(kb + 1) * 128],
                                wv_sb[:, c, :],
                                start=(c == 0), stop=(c == NC_CHUNKS - 1))
                        # scatter per-head 64-wide slices (one strided copy)
                        nc.vector.tensor_copy(
                            v_sb[:, kb, :, 0:64],
                            ps[:].rearrange("p (h d) -> p h d", h=HPC))

                # ---- attention ----
                with tc.tile_pool(name="aps", bufs=1, space="PSUM") as aps:
                    for h in range(HPC):
                        hp = 64 * (h % 2)
                        hc = h // 2
                        out_ps = [aps.tile([128, QB], f32, name=f"out_ps{qb}",
                                           tag=f"ops{qb}", bufs=1)
                                  for qb in range(NQB)]
                        for kb in range(NKB):
                            s_big = aps.tile([128, NQB, QB], f32, name="s_big",
                                             tag="sbig", bufs=1)
                            for qb in range(NQB):
                                nc.tensor.matmul(
                                    s_big[:, qb, :],
                                    kT_sb[hp:hp + 64, hc, kb * 128:(kb + 1) * 128],
                                    qT_sb[hp:hp + 64, hc, qb * QB:(qb + 1) * QB],
                                    start=True, stop=True)
                            pT = workpool.tile([128, NQB * QB], f16, name="pT",
                                               tag="pT", bufs=2)
                            nc.scalar.activation(
                                pT[:], s_big[:].rearrange("p a b -> p (a b)"),
                                EXP, scale=0.125)
                            for qb in range(NQB):
                                nc.tensor.matmul(
                                    out_ps[qb][0:65, :],
                                    v_sb[:, kb, h, 0:65],
                                    pT[:, qb * QB:(qb + 1) * QB],
                                    start=(kb == 0), stop=(kb == NKB - 1))
                        # normalize: out^T rows 0:64 divided by row 64 (denoms)
                        for qb in range(NQB):
                            recip = workpool.tile([1, QB], f32, name="recip",
                                                  tag="recip", bufs=2)
                            nc.vector.reciprocal(recip[:], out_ps[qb][64:65, :])
                            rbc = workpool.tile([64, QB], f32, name="rbc",
                                                tag="rbc", bufs=2)
                            nc.gpsimd.partition_broadcast(rbc[:], recip[0:1, :])
                            nc.vector.tensor_mul(
                                oT_sb[hp:hp + 64, hc, qb * QB:(qb + 1) * QB],
                                out_ps[qb][0:64, :], rbc[:])

                # ---- output projection ----
                with tc.tile_pool(name="yps", bufs=2, space="PSUM") as yps:
                    for t in range(NKB):
                        y_sb = workpool.tile([128, D], f32, name="y_sb",
                                             tag="y", bufs=2)
                        for n2 in range(2):
                            ps = yps.tile([128, QB], f32, name="y_ps", tag="yps")
                            for c2 in range(4):
                                nc.tensor.matmul(
                                    ps[:],
                                    oT_sb[:, c2, t * 128:(t + 1) * 128],
                                    wo_sb[:, c2, n2 * QB:(n2 + 1) * QB],
                                    start=(c2 == 0), stop=(c2 == 3))
                            nc.vector.tensor_copy(y_sb[:, n2 * QB:(n2 + 1) * QB],
                                                  ps[:])
                        nc.sync.dma_start(out=yp_d[t * 128:(t + 1) * 128, :],
                                          in_=y_sb[:])

    nc.compile()
    _prog_cache[reps] = nc
    return nc


def _chunk_pT(x):
    """[S, D] -> [128, D//128, S] fp16 (X^T chunked: out[p, c, t] = x[t, 128c+p])."""
    a = np.ascontiguousarray(x.reshape(S, NC_CHUNKS, 128).transpose(2, 1, 0))
    return a


def _chunk_w(w):
    """[D, GCOLS] -> [128, 8, GCOLS]: out[p, c, m] = w[128c+p, m]."""
    return np.ascontiguousarray(
        w.reshape(NC_CHUNKS, 128, w.shape[1]).transpose(1, 0, 2))


def prepare_in_maps(Q, K, V, W_Q, W_K, W_V, W_O):
    f16 = np.float16
    qt = [_chunk_pT(Q[b].astype(f16)) for b in range(B)]
    kt = [_chunk_pT(K[b].astype(f16)) for b in range(B)]
    vt = [_chunk_pT(V[b].astype(f16)) for b in range(B)]
    wq = [_chunk_w(W_Q[:, g * GCOLS:(g + 1) * GCOLS].astype(f16)) for g in range(2)]
    wk = [_chunk_w(W_K[:, g * GCOLS:(g + 1) * GCOLS].astype(f16)) for g in range(2)]
    wv = [_chunk_w(W_V[:, g * GCOLS:(g + 1) * GCOLS].astype(f16)) for g in range(2)]
    # wo rows for group g, chunked: [128, 4, D]
    wo = [np.ascontiguousarray(
        W_O[g * GCOLS:(g + 1) * GCOLS, :].astype(f16)
        .reshape(4, 128, D).transpose(1, 0, 2)) for g in range(2)]
    in_maps = []
    for c in range(NCORES):
        b, g = c // 2, c % 2
        in_maps.append({
            "qt": qt[b], "kt": kt[b], "vt": vt[b],
            "wq": wq[g], "wk": wk[g], "wv": wv[g], "wo": wo[g],
        })
    return in_maps


def execute(nc, in_maps):
    from concourse.bass_utils import run_bass_kernel_spmd
    res = run_bass_kernel_spmd(nc, in_maps, list(range(NCORES)))
    return res


def _numpy_fallback(Q, K, V, mask, W_Q, W_K, W_V, W_O):
    import math
    B_, S1, _ = Q.shape
    q = (Q.reshape(-1, D) @ W_Q).reshape(B_, S1, H, DK).transpose(0, 2, 1, 3)
    k = (K.reshape(-1, D) @ W_K).reshape(B_, S1, H, DK).transpose(0, 2, 1, 3)
    v = (V.reshape(-1, D) @ W_V).reshape(B_, S1, H, DK).transpose(0, 2, 1, 3)
    out = np.empty((B_, H, S1, DK), np.float32)
    for b in range(B_):
        for h in range(H):
            s = (q[b, h] @ k[b, h].T) / math.sqrt(DK)
            s = np.where(mask[b] == 0, np.float32(-1e9), s)
            s = s - s.max(axis=-1, keepdims=True)
            e = np.exp(s)
            p = e / e.sum(axis=-1, keepdims=True)
            out[b, h] = p @ v[b, h]
    o = out.transpose(0, 2, 1, 3).reshape(B_, S1, D)
    return (o.reshape(-1, D) @ W_O).reshape(B_, S1, D).astype(np.float32)


def kernel(Q, K, V, mask, W_Q, W_K, W_V, W_O):
    Q = np.asarray(Q); K = np.asarray(K); V = np.asarray(V)
    mask = np.asarray(mask)
    W_Q = np.asarray(W_Q); W_K = np.asarray(W_K)
    W_V = np.asarray(W_V); W_O = np.asarray(W_O)
    if (mask == 0).any():
        # spec guarantees an all-ones mask; this path is correctness insurance
        return _numpy_fallback(Q, K, V, mask, W_Q, W_K, W_V, W_O)
    nc = build_program()
    in_maps = prepare_in_maps(Q, K, V, W_Q, W_K, W_V, W_O)
    res = execute(nc, in_maps)
    out = np.empty((B, S, D), np.float32)
    for b in range(B):
        out[b] = res.results[2 * b]["yp"] + res.results[2 * b + 1]["yp"]
    return out
